# revision 74
# baseline (speedup 1.0000x reference)
"""RNN-T Joint network kernel for Trainium2 (Bass/Tile), 8-core data-parallel.

Math (per batch b):
  hf = f[b] @ W1[:1024]            # (T=256, J=640)
  hg = g[b] @ W1[1024:]            # (U=65,  J=640)
  h[t,u,:]   = relu(hf[t] + hg[u] + b1)
  out[t,u,:] = h[t,u,:] @ W2 + b2  # (256, 65, 1024)

Sharding: data-parallel over B=8, one utterance per core.  Host-side prep
(part of the sharding step): inputs are cast/packed into partition-major
layouts; weights for the fp8 path are pre-split hi/lo; outputs come back
fp16 and are upcast on the host.

Device schedule (per core, u-major):
  - hfT[j, t] (f32) and hgT'[j, u] = hgT + b1 (f32, x0.25 for fp8 tiles)
    resident in SBUF (j on partitions), computed by PE (bf16 operands).
  - The layer-2 contraction (J=640 = 5 k-tiles of 128) is split: the first
    (5-K8) tiles stay bf16; the last K8 tiles use fp8e4m3 with DoubleRow
    perf mode (2 k-tiles per PE pass at 0.5 cycles/row).  To keep accuracy,
    H is quantized once per tile (relu act emits fp8 directly, scale 1/4)
    while W2 is pre-split into hi+lo fp8 pairs (scale 4) so its
    quantization error cancels to ~bf16 level.  Measured end-to-end max
    rel err ~1.6-1.9e-2 scale vs the 2e-2 gate (K8=3 / K8=4).
  - Per u: ScalarE builds H tiles (bf16) and the fp8 slot tile, PE runs
    2x2 (tt x vh) PSUM groups of [128,512], DVE drains psum + b2 into
    fp16 out tiles, one DMA per (u, tt) straight to HBM.
"""

import numpy as np

T, U = 256, 65
EH, PH, J, V = 1024, 320, 640, 1024
JC = J // 128           # 5 j-chunks
HC = EH // 128          # 8 h-chunks (f side)
GC = 3                  # g-side chunks (PH padded 320 -> 384 = 3*128)
N_CORES = 8

K8 = 4                  # number of fp8 k-tiles (from the top); 5-K8 stay bf16
NBF = JC - K8

_CACHE = {}


def _build_nc():
    import concourse.bass as bass
    import concourse.bacc as bacc
    import concourse.mybir as mybir
    from concourse import tile

    f32 = mybir.dt.float32
    f16 = mybir.dt.float16
    bf16 = mybir.dt.bfloat16
    e4 = mybir.dt.float8e4
    Relu = mybir.ActivationFunctionType.Relu
    add = mybir.AluOpType.add
    mult = mybir.AluOpType.mult
    DR = mybir.MatmulPerfMode.DoubleRow

    nc = bacc.Bacc(None, target_bir_lowering=False)

    # packed, partition-major inputs (see _pack_* helpers); gside is
    # [gT | b1 | W1g] merged into one array -> one DMA
    GSIDE = GC * U + JC + JC * GC * 128
    # fp8 W2 pair tiles: K8=4 -> (hi12, lo12, hi34, lo34); K8=3 ->
    # (hi23, lo23, hi4lo4)
    NPAIR = K8 if K8 == 4 else 3
    fT_d = nc.declare_dram_parameter("fTp", [128, HC * T], bf16, isOutput=False)
    gs_d = nc.declare_dram_parameter("gside", [128, GSIDE], bf16, isOutput=False)
    W1f_d = nc.declare_dram_parameter("W1fp", [128, JC * HC * 128], bf16,
                                      isOutput=False)
    W2b_d = nc.declare_dram_parameter("W2bf", [128, NBF * V], bf16,
                                      isOutput=False)
    W2q_d = nc.declare_dram_parameter("W2q", [128, NPAIR, 2, V], e4,
                                      isOutput=False)
    b2_d = nc.declare_dram_parameter("b2p", [1, V], bf16, isOutput=False)
    out_d = nc.declare_dram_parameter("out", [T, U, V], f16, isOutput=True)

    with tile.TileContext(nc) as tc:
        with tc.tile_pool(name="const", bufs=1) as cpool:
            # Preload the ScalarE activation table (Relu) off the critical
            # path: the first act instruction pays ~1.3us table load.
            dumin = cpool.tile([128, 1], f32)
            nc.gpsimd.memset(dumin[:], 0.0)
            dumout = cpool.tile([128, 1], f32)
            nc.scalar.activation(dumout[:], dumin[:], Relu, bias=0.0, scale=1.0)

            # ---------------- DMA (priority order) ----------------
            # Few large DMAs via SP/HWDGE, ordered so each PE consumer's
            # input lands just before the (ramp-paced) PE stream reaches it.
            fTall = cpool.tile([128, HC * T], bf16)
            W1fall = cpool.tile([128, JC * HC * 128], bf16)
            gsall = cpool.tile([128, GSIDE], bf16)
            W2ball = (cpool.tile([128, NBF * V], bf16, name="W2ball")
                      if NBF else None)
            # one 3D tile per fp8 pair: [128, 2(k-tiles), V] — DoubleRow
            # needs the k-pair dim at AP position 1
            W2qp = [cpool.tile([128, 2, V], e4, name=f"W2qp{p}")
                    for p in range(NPAIR)]
            b2row = cpool.tile([1, V], bf16)
            b2bc = cpool.tile([128, V], f32)

            half_f = 5 * T
            half_w = 5 * 128

            def dma_w1f(c):
                nc.sync.dma_start(
                    out=W1fall[:, c * HC * 128:(c + 1) * HC * 128],
                    in_=W1f_d[:, c * HC * 128:(c + 1) * HC * 128])

            def dma_w2q(p, vh=None, eng=None):
                # vh halves let the first consumer start after half the
                # pair has landed; eng=gpsimd moves desc-gen to the Pool
                # engine's private SWDGE path (off the serial HWDGE chain)
                if vh is None:
                    (eng or nc.sync).dma_start(
                        out=W2qp[p][:], in_=W2q_d[:, p, :, :])
                else:
                    (eng or nc.sync).dma_start(
                        out=W2qp[p][:, :, vh * 512:(vh + 1) * 512],
                        in_=W2q_d[:, p, :, vh * 512:(vh + 1) * 512])

            nc.sync.dma_start(out=fTall[:, :half_f], in_=fT_d[:, :half_f])
            nc.sync.dma_start(out=W1fall[:, :half_w], in_=W1f_d[:, :half_w])
            # tiny b2 row via Pool/SWDGE: off the HWDGE ladder, and its
            # 11ns transfer jumping the DMA queue is harmless; PE
            # broadcasts it across partitions during the prologue.
            nc.gpsimd.dma_start(out=b2row[:], in_=b2_d[:])
            dma_w1f(1)
            nc.sync.dma_start(out=W1fall[:, half_w:HC * 128],
                              in_=W1f_d[:, half_w:HC * 128])
            nc.sync.dma_start(out=fTall[:, half_f:], in_=fT_d[:, half_f:])
            dma_w1f(2)
            nc.sync.dma_start(out=gsall[:], in_=gs_d[:])
            for c in range(NBF):
                nc.sync.dma_start(out=W2ball[:, c * V:(c + 1) * V],
                                  in_=W2b_d[:, c * V:(c + 1) * V])
            dma_w1f(3)
            dma_w2q(0)
            dma_w2q(1)
            dma_w1f(4)
            for p in range(2, NPAIR):
                dma_w2q(p)

            def fT(h):
                return fTall[:, h * T:(h + 1) * T]

            def w1f(c, h):
                o = (c * HC + h) * 128
                return W1fall[:, o:o + 128]

            def gT(pc):
                return gsall[:, pc * U:(pc + 1) * U]

            b1bf = gsall[:, GC * U:GC * U + JC]
            b1sb = cpool.tile([128, JC], f32)

            def w1g(c, pc):
                o = GC * U + JC + (c * GC + pc) * 128
                return gsall[:, o:o + 128]

            def w2b(c, lo, hi):
                return W2ball[:, c * V + lo:c * V + hi]

            # ------------- single gapless PE stream -------------
            # One PSUM pool (2 tags x 2 bufs x [128,1024]f32 two-bank tiles
            # = all 8 banks).  Both vh groups of a tt live in one tile so
            # the drain is a single [128,1024] DVE op (1192ns vs 2x658):
            # with the fp8-shortened PE stream (2560ns/u) two 658ns drains
            # per tt would make DVE the bottleneck.
            hfTs = [None] * JC   # f32 [128, T]  (hf^T)
            hgTs = [None] * JC   # f32 [128, U]  (hg^T + b1; x0.25 on fp8 tiles)

            with (
                tc.tile_pool(name="hpool", bufs=4) as hpool,
                tc.tile_pool(name="opool", bufs=3) as opool,
                tc.tile_pool(name="mpsum", bufs=4, space=bass.MemorySpace.PSUM) as mpsum,
            ):
                # single-tag 4-deep rotation over the 4 two-bank tiles: each
                # reuse is 4 allocations back, which lets u0 and u1 hold
                # groups open simultaneously during the interleaved prologue
                def psum(tag="ps"):
                    return mpsum.tile([128, 1024], f32, tag=tag,
                                      name=f"ps_{tag}")

                def hfT_mms(pf, col0, c, h0, h1):
                    for h in range(h0, h1):
                        nc.tensor.matmul(pf[:, col0:col0 + T], w1f(c, h),
                                         fT(h),
                                         start=(h == 0), stop=(h == HC - 1))

                def hfT_drain(pf, col0, c):
                    t = cpool.tile([128, T], f32, tag=f"hfT{c}")
                    nc.vector.tensor_copy(t[:], pf[:, col0:col0 + T])
                    hfTs[c] = t

                def hfT_chunk(c, pf, col0):
                    hfT_mms(pf, col0, c, 0, HC)
                    hfT_drain(pf, col0, c)

                def hgT_chunk(c, ph, col0):
                    # hg^T[c] = sum_pc W1g[c][pc].T @ gT[pc]  (+ b1 on drain;
                    # fp8 tiles also fold the 1/4 act pre-scale in here)
                    for pc in range(GC):
                        nc.tensor.matmul(ph[:, col0:col0 + U], w1g(c, pc),
                                         gT(pc),
                                         start=(pc == 0), stop=(pc == GC - 1))
                    t = cpool.tile([128, U], f32, tag=f"hgT{c}")
                    if c >= NBF:
                        nc.vector.tensor_scalar(t[:], ph[:, col0:col0 + U],
                                                b1sb[:, c:c + 1], 0.25,
                                                add, mult)
                    else:
                        nc.vector.tensor_scalar(t[:], ph[:, col0:col0 + U],
                                                b1sb[:, c:c + 1], None, add)
                    hgTs[c] = t

                # H tiles for u: bf16 tiles (c < NBF) separate; fp8 tiles
                # in one [128, 4, T] slot tile (K8=3 duplicates tile 4 in
                # slots 2,3 so DoubleRow pairs stay free-dim-adjacent).
                SLOT_C = [1, 2, 3, 4] if K8 == 4 else [2, 3, 4, 4]

                def act_u(u, slots=range(4), hb_hq=None):
                    if hb_hq is None:
                        Hb = []
                        for c in range(NBF):
                            ht = hpool.tile([128, T], bf16, tag=f"H{c}")
                            nc.scalar.activation(ht[:], hfTs[c][:], Relu,
                                                 bias=hgTs[c][:, u:u + 1],
                                                 scale=1.0)
                            Hb.append(ht)
                        hq = hpool.tile([128, 4, T], e4, tag="HQ")
                    else:
                        Hb, hq = hb_hq
                    for si in slots:
                        c = SLOT_C[si]
                        nc.scalar.activation(hq[:, si, :], hfTs[c][:], Relu,
                                             bias=hgTs[c][:, u:u + 1],
                                             scale=0.25)
                    return Hb, hq

                # Matmul sequence per (tt, vh) psum group: NBF bf16 k-tiles
                # then the fp8 DoubleRow pairs.  W2qall pair p holds
                # [128, 2, V]; K8=4: p0=hi(1,2) p1=lo(1,2) p2=hi(3,4)
                # p3=lo(3,4); K8=3: p0=hi(2,3) p1=lo(2,3) p2=(hi4,lo4).
                if K8 == 4:
                    DR_SEQ = [(0, 0), (0, 1), (2, 2), (2, 3)]  # (slot0, pair)
                else:
                    DR_SEQ = [(0, 0), (0, 1), (2, 2)]
                NMM = NBF + len(DR_SEQ)

                def group_mms(specs, Hb, hq, tt, mi, stop=True):
                    # emit matmul index mi for each (psumAP, vlo, vwid)
                    # group; stop=False keeps the group open past the last
                    # index (the caller appends its own stopping matmul)
                    ts = slice(tt * 128, (tt + 1) * 128)
                    last = stop and mi == NMM - 1
                    for ps, lo, wid in specs:
                        if mi < NBF:
                            nc.tensor.matmul(
                                ps, Hb[mi][:, ts], w2b(mi, lo, lo + wid),
                                start=(mi == 0), stop=last)
                        else:
                            s0, p = DR_SEQ[mi - NBF]
                            nc.tensor.matmul(
                                ps, hq[:, s0:s0 + 2, ts],
                                W2qp[p][:, :, lo:lo + wid],
                                start=(mi == 0), stop=last,
                                perf_mode=DR)

                dums = cpool.tile([1, 512], bf16)
                nc.vector.memset(dums[:], 0.0)
                ones = cpool.tile([1, 128], bf16)
                nc.vector.memset(ones[:], 1.0)

                # PE warm-up chain: dummy matmuls gated only on two tiny
                # DVE memsets keep the PE busy-clock running through the
                # initial DMA wait, so the cost model's p-state ramp (~3us
                # of 2x-slow after any idle) is spent on throwaway work and
                # the real stream runs at full rate.  The last two slots
                # broadcast b2 across partitions (ready ~3.8us via the tiny
                # SWDGE b2row DMA), replacing a 1.4us-wide b2bc input DMA
                # on the critical input ladder.
                warm = psum()
                for wd in range(5):
                    nc.tensor.matmul(warm[0:1, 0:512], dums[0:1, 0:1],
                                     dums[0:1, :], start=True, stop=True)
                for vh in range(2):
                    pb = warm[:, vh * 512:(vh + 1) * 512]
                    nc.tensor.matmul(pb, ones[:],
                                     b2row[:, vh * 512:(vh + 1) * 512],
                                     start=True, stop=True)
                    nc.vector.tensor_copy(b2bc[:, vh * 512:(vh + 1) * 512],
                                          pb)

                # hfT chunks 0-2, hgT chunks, then u0 with skewed stages.
                # Layer-1 psums use halves of the two-bank tiles; the pool's
                # 2-buf rotation keeps reuse two allocations apart.
                pf0 = psum()
                hfT_mms(pf0, 0, 0, 0, 5)
                pf1 = psum()
                hfT_mms(pf1, 0, 1, 0, 5)
                hfT_mms(pf0, 0, 0, 5, HC)
                hfT_drain(pf0, 0, 0)
                hfT_mms(pf1, 0, 1, 5, HC)
                hfT_drain(pf1, 0, 1)
                pq = psum()
                hfT_chunk(2, pq, 0)
                nc.vector.tensor_copy(b1sb[:], b1bf)
                hgT_chunk(0, pq, 512)
                ph1 = psum()
                hgT_chunk(1, ph1, 0)
                hgT_chunk(2, ph1, 512)
                pr = psum()
                hgT_chunk(3, pr, 0)
                hgT_chunk(4, pr, 512)
                assert K8 == 4, "u0 early/late act split assumes K8=4 tiling"
                # u0 acts for the early tiles only: DoubleRow pairs (1,2)
                # need just hfT c1/c2, so u0's first three matmul stages can
                # run while W1f c3/c4 and the late W2q pairs still stream in;
                # hfT c3/c4 + the remaining acts slot in mid-u0, and u1's
                # first stages cover the last W2q pair deadlines.
                Hb0, hq0 = act_u(0, slots=(0, 1))

                # u0 skewed stages: tt0 groups run matmul-index mi while
                # tt1 groups run mi-1, giving input DMAs extra lead time.
                G0 = psum()
                sp0 = [(G0[:, 0:512], 0, 512), (G0[:, 512:1024], 512, 512)]
                group_mms(sp0, Hb0, hq0, 0, 0)
                pX = psum()
                hfT_chunk(3, pX, 0)
                hfT_chunk(4, pX, 512)
                act_u(0, slots=(2, 3), hb_hq=(Hb0, hq0))
                G1 = psum()
                sp1 = [(G1[:, 0:512], 0, 512), (G1[:, 512:1024], 512, 512)]
                for mi in (1, 2):
                    group_mms(sp0, Hb0, hq0, 0, mi)
                    group_mms(sp1, Hb0, hq0, 1, mi - 1)
                # u1's acts + first stages slot in here: they only need
                # W2bf/W2q0/W2q1, buying ~1.3us of PE work before u0's
                # mi3/mi4 (the late W2q2/W2q3 consumers) come due.
                Hb1, hq1 = act_u(1)
                U1 = psum()
                su1 = [(U1[:, 0:512], 0, 512), (U1[:, 512:1024], 512, 512)]
                for mi in (0, 1, 2):
                    group_mms(su1, Hb1, hq1, 0, mi)
                for mi in (3, 4):
                    group_mms(sp0, Hb0, hq0, 0, mi)
                    group_mms(sp1, Hb0, hq0, 1, mi - 1)
                ot0 = opool.tile([128, V], f16, tag="o0")
                nc.vector.tensor_tensor(ot0[:], G0[:], b2bc[:], add)
                nc.sync.dma_start(out=out_d[0:128, 0, :], in_=ot0[:])
                group_mms(sp1, Hb0, hq0, 1, NMM - 1)
                ot1 = opool.tile([128, V], f16, tag="o1")
                nc.vector.tensor_tensor(ot1[:], G1[:], b2bc[:], add)
                nc.gpsimd.dma_start(out=out_d[128:256, 0, :], in_=ot1[:])
                # finish u1: tt0's last stages, then tt1 in the usual shape
                for mi in (3, 4):
                    group_mms(su1, Hb1, hq1, 0, mi)
                otu1 = opool.tile([128, V], f16, tag="o0")
                nc.vector.tensor_tensor(otu1[:], U1[:], b2bc[:], add)
                nc.sync.dma_start(out=out_d[0:128, 1, :], in_=otu1[:])
                U1b = psum()
                su1b = [(U1b[:, 0:512], 0, 512), (U1b[:, 512:1024], 512, 512)]
                for mi in range(NMM):
                    group_mms(su1b, Hb1, hq1, 1, mi)
                otu1b = opool.tile([128, V], f16, tag="o1")
                nc.vector.tensor_tensor(otu1b[:], U1b[:], b2bc[:], add)
                nc.gpsimd.dma_start(out=out_d[128:256, 1, :], in_=otu1b[:])

                # ---------------- u = 2 .. U-2 ----------------
                for u in range(2, U - 1):
                    Hb, hq = act_u(u)
                    for tt in range(2):
                        big = psum()
                        sp = [(big[:, 0:512], 0, 512),
                              (big[:, 512:1024], 512, 512)]
                        for mi in range(NMM):
                            group_mms(sp, Hb, hq, tt, mi)
                        ot = opool.tile([128, V], f16, tag=f"o{tt}")
                        nc.vector.tensor_tensor(ot[:], big[:], b2bc[:], add)
                        # tt1 stores go out via Pool/SWDGE: its descriptor
                        # generation runs on the idle Pool engine instead of
                        # the shared HWDGE, halving both the SP queue load
                        # (~1.3us/DMA) and the HWDGE ladder.
                        eng = nc.sync if tt == 0 else nc.gpsimd
                        eng.dma_start(
                            out=out_d[tt * 128:(tt + 1) * 128, u, :],
                            in_=ot[:])

                # ---------------- final u: short tail ----------------
                # tt=0 keeps the steady shape.  tt=1 splits: bank A is one
                # [128,512] group; bank B runs two sequential [128,256]
                # quarter-groups (one active group per 2KB zero region),
                # so the very last drain+store chain is short.  The last
                # store uses the otherwise-idle Activation HWDGE queue.
                # tt0 keeps the steady shape; tt1 splits its last 512
                # columns across two additional psum BANKS (one from a
                # second a0 allocation) so the three groups interleave
                # without zero-region conflicts and the very last
                # drain+store chain is a short [128,256] piece on the
                # otherwise-idle Activation HWDGE queue.
                u = U - 1
                Hb, hq = act_u(u)
                big = psum()
                sp = [(big[:, 0:512], 0, 512), (big[:, 512:1024], 512, 512)]
                for mi in range(NMM):
                    group_mms(sp, Hb, hq, 0, mi)
                ot = opool.tile([128, V], f16, tag="o0")
                nc.vector.tensor_tensor(ot[:], big[:], b2bc[:], add)
                nc.sync.dma_start(out=out_d[0:128, u, :], in_=ot[:])

                big1 = psum()
                bigX = psum()
                spA = [(big1[:, 0:512], 0, 512),
                       (big1[:, 512:768], 512, 256)]
                for mi in range(NMM):
                    group_mms(spA, Hb, hq, 1, mi)
                    group_mms([(bigX[:, 0:256], 768, 256)], Hb, hq, 1, mi,
                              stop=False)
                # PE adds b2 into the bigX piece so ScalarE can drain it
                # with a plain Copy activation — in PARALLEL with the DVE
                # drains of the other two pieces; three store chains on
                # three queues.
                nc.tensor.matmul(bigX[:, 0:256], ones[:],
                                 b2row[:, 768:1024], start=False, stop=True)
                ot1 = opool.tile([128, V], f16, tag="o1")
                nc.scalar.activation(ot1[:, 768:1024], bigX[:, 0:256],
                                     mybir.ActivationFunctionType.Copy,
                                     bias=0.0, scale=1.0)
                nc.scalar.dma_start(out=out_d[128:256, u, 768:1024],
                                    in_=ot1[:, 768:1024])
                nc.vector.tensor_tensor(ot1[:, 0:512], big1[:, 0:512],
                                        b2bc[:, 0:512], add)
                nc.sync.dma_start(out=out_d[128:256, u, 0:512],
                                  in_=ot1[:, 0:512])
                nc.vector.tensor_tensor(ot1[:, 512:768], big1[:, 512:768],
                                        b2bc[:, 512:768], add)
                nc.gpsimd.dma_start(out=out_d[128:256, u, 512:768],
                                    in_=ot1[:, 512:768])
    nc.compile()
    return nc


def _get_nc():
    if "nc" not in _CACHE:
        _CACHE["nc"] = _build_nc()
    return _CACHE["nc"]


def _pack_shared(W1, b1, W2, b2):
    """Partition-major packed weights, shared across cores."""
    import ml_dtypes

    bf16 = ml_dtypes.bfloat16
    e4 = ml_dtypes.float8_e4m3fn
    W1 = np.asarray(W1, dtype=np.float32)
    W2 = np.asarray(W2, dtype=np.float32)
    # W1f: [p, c, h, k] with source index [h*128+p, c*128+k]
    W1fp = np.ascontiguousarray(
        W1[:EH].reshape(HC, 128, JC, 128).transpose(1, 2, 0, 3)
        .reshape(128, JC * HC * 128)).astype(bf16)
    # W1g: pad rows to 384, then [p, c, pc, k]
    W1g = np.zeros((GC * 128, J), dtype=np.float32)
    W1g[:PH] = W1[EH:]
    W1gp = np.ascontiguousarray(
        W1g.reshape(GC, 128, JC, 128).transpose(1, 2, 0, 3)
        .reshape(128, JC * GC * 128)).astype(bf16)
    # bf16 W2 k-tiles: [p, c, v] with source [c*128+p, v]
    W2t = W2.reshape(JC, 128, V).transpose(1, 0, 2)  # [p, c, v]
    W2bf = np.ascontiguousarray(W2t[:, :NBF].reshape(128, NBF * V)).astype(bf16)
    # fp8 tiles: hi/lo split at scale x4 (exact power of two; the act
    # pre-scales H by 1/4 so no drain-side compensation is needed)
    ws = W2t[:, NBF:].astype(bf16).astype(np.float32) * 4.0  # [p, K8, V]
    w_hi = ws.astype(e4)
    w_lo = (ws - w_hi.astype(np.float32)).astype(e4)
    if K8 == 4:
        pairs = [
            np.stack([w_hi[:, 0], w_hi[:, 1]], axis=1),
            np.stack([w_lo[:, 0], w_lo[:, 1]], axis=1),
            np.stack([w_hi[:, 2], w_hi[:, 3]], axis=1),
            np.stack([w_lo[:, 2], w_lo[:, 3]], axis=1),
        ]
    else:
        pairs = [
            np.stack([w_hi[:, 0], w_hi[:, 1]], axis=1),
            np.stack([w_lo[:, 0], w_lo[:, 1]], axis=1),
            np.stack([w_hi[:, 2], w_lo[:, 2]], axis=1),
        ]
    W2qp = np.ascontiguousarray(np.stack(pairs, axis=1))  # [128, NPAIR, 2, V]
    b1p = np.asarray(b1, dtype=np.float32).reshape(JC, 128).T.astype(bf16)
    b2p = np.ascontiguousarray(
        np.asarray(b2, dtype=np.float32).reshape(1, V)).astype(bf16)
    return W1fp, W1gp, W2bf, W2qp, b1p, b2p


def _pack_core(f_b, g_b, b1p, W1gp):
    import ml_dtypes

    bf16 = ml_dtypes.bfloat16
    # fT packed: [p, h, t] with source f[t, h*128+p]
    fTp = np.ascontiguousarray(
        f_b.T.reshape(HC, 128, T).transpose(1, 0, 2).reshape(128, HC * T)
    ).astype(bf16)
    # gT packed: pad rows of g^T [PH, U] to 384 = GC*128
    gTfull = np.zeros((GC * 128, U), dtype=np.float32)
    gTfull[:PH] = g_b.T
    gTp = (gTfull.reshape(GC, 128, U).transpose(1, 0, 2)
           .reshape(128, GC * U).astype(bf16))
    # one merged g-side array: [gT | b1 | W1g]
    gside = np.ascontiguousarray(np.concatenate([gTp, b1p, W1gp], axis=1))
    return fTp, gside


def run(f, g, W1, b1, W2, b2, trace=False):
    """Returns (full_output, BassKernelResults)."""
    from concourse.bass_utils import run_bass_kernel_spmd

    nc = _get_nc()

    W1fp, W1gp, W2bf, W2qp, b1p, b2p = _pack_shared(W1, b1, W2, b2)
    f = np.asarray(f, dtype=np.float32)
    g = np.asarray(g, dtype=np.float32)

    in_maps = []
    for i in range(N_CORES):
        fTp, gside = _pack_core(f[i], g[i], b1p, W1gp)
        in_maps.append({
            "fTp": fTp,
            "gside": gside,
            "W1fp": W1fp,
            "W2bf": W2bf,
            "W2q": W2qp,
            "b2p": b2p,
        })
    res = run_bass_kernel_spmd(nc, in_maps, list(range(N_CORES)), trace=trace)
    out = np.stack([np.asarray(res.results[i]["out"], dtype=np.float32)
                    for i in range(N_CORES)], axis=0)
    return out, res


def kernel(f, g, W1, b1, W2, b2):
    out, _ = run(f, g, W1, b1, W2, b2)
    return out
